# revision 23
# baseline (speedup 1.0000x reference)
"""Trainium2 Bass kernel for CLSProcess: diagonal linear recurrence
state_t = y_t * state_{t-1} + x_t * z_t over [B=8, T=4096, units=1024].

Sharding: batch across the 8 cores (one batch element per core); the
recurrence is handled per-core with a chunked scan:
  - time is cut into 32 blocks of L=128 steps (partition dim = time)
  - per block, the decay matrix M[t,s] = prod_{r=s+1..t} y_r (0 for s>t)
    is built EXACTLY with a DVE tensor_tensor_scan over the identity:
    state_s(t) = y_t*state + I[s==t]  =>  out[s,t] = M[t,s] (the lhsT
    layout the PE matmul wants). Scans are batched 4 blocks per
    instruction ([128,512]) with the y at block boundaries zeroed so the
    running state resets at each block start.
  - block output = M @ (x*z)  (PE matmul, bf16 operands, fp32 PSUM) +
    carry term
  - carry term: engines can only address partition bases {0,32,64,96},
    so instead of extracting row 127 of the previous block we build
    sel[s,t] = I[s==127] * p_t  (p_t = prod_{r=block_start..t} y_r
    = y_0 * M[t,0], broadcast via GPSIMD + mask on DVE) and accumulate
    sel^T @ prev_out into the same PSUM (float32r single-pass matmul),
    which equals p_t * prev_state.
"""

import numpy as np

import concourse.bacc as bacc
import concourse.bass as bass
import concourse.mybir as mybir
import concourse.tile as tile
from concourse.bass_utils import run_bass_kernel_spmd

B = 8
T = 4096
F = 1026
U = 1024
L = 128
G = 4  # blocks per scan batch
f32 = mybir.dt.float32
f32r = mybir.dt.float32r
bf16 = mybir.dt.bfloat16


def build_nc(t_total: int = T) -> bass.Bass:
    nb = t_total // L
    ng = (nb + G - 1) // G
    nc = bacc.Bacc()
    inp = nc.dram_tensor("inp", [t_total, F], f32, kind="ExternalInput")
    out = nc.dram_tensor("out", [t_total, U], f32, kind="ExternalOutput")
    ident_d = nc.inline_tensor(np.eye(L, dtype=np.float32), name="ident")
    ident4_d = nc.inline_tensor(
        np.tile(np.eye(L, dtype=np.float32), (1, G)), name="ident4"
    )
    e127c_np = np.zeros((L, 1), dtype=np.float32)
    e127c_np[L - 1, 0] = 1.0
    e127c_d = nc.inline_tensor(e127c_np, name="e127c")

    with tile.TileContext(nc) as tc:
        with (
            tc.tile_pool(name="const", bufs=1) as constp,
            tc.tile_pool(name="inpool", bufs=12) as inpool,
            tc.tile_pool(name="upool", bufs=4) as upool,
            tc.tile_pool(name="mpool", bufs=3) as mpool,
            tc.tile_pool(name="rowpool", bufs=3) as rowpool,
            tc.tile_pool(name="prowpool", bufs=3) as prowpool,
            tc.tile_pool(name="bcpool", bufs=3) as bcpool,
            tc.tile_pool(name="pbcpool", bufs=3) as pbcpool,
            tc.tile_pool(name="selpool", bufs=3) as selpool,
            tc.tile_pool(name="outpool", bufs=4) as outpool,
            tc.tile_pool(name="carrypool", bufs=3) as carrypool,
            tc.tile_pool(name="ps_small", bufs=2, space="PSUM") as ps_small_pool,
            tc.tile_pool(name="ps_out", bufs=3, space="PSUM") as ps_out_pool,
        ):
            ident = constp.tile([L, L], f32, tag="ident")
            nc.sync.dma_start(ident[:], ident_d[:, :])
            ident4 = constp.tile([L, G * L], f32, tag="ident4")
            nc.sync.dma_start(ident4[:], ident4_d[:, :])
            e127c = constp.tile([L, 1], f32, tag="e127c")
            nc.sync.dma_start(e127c[:], e127c_d[:, :])

            prev = None
            tins = {}
            for g in range(ng):
                ks = list(range(g * G, min((g + 1) * G, nb)))
                # per-group y rows: yrow4[0, L*j + i] = y(block ks[j], step i),
                # with the block-start column zeroed (scan state reset)
                yrow4 = rowpool.tile([1, G * L], f32, tag="yrow4")
                nc.vector.memset(yrow4[:], 0.0)
                for j, k in enumerate(ks):
                    r0 = k * L
                    tin = inpool.tile([L, F], f32, tag="tin")
                    nc.sync.dma_start(tin[:], inp[r0 : r0 + L, :])
                    tins[k] = tin
                    ps = ps_small_pool.tile([1, L], f32, tag="ps_small")
                    nc.tensor.transpose(ps[0:1, :], tin[:, 1:2], ident[:])
                    nc.scalar.copy(yrow4[0:1, L * j + 1 : L * j + L], ps[0:1, 1:L])
                ybc4 = bcpool.tile([L, G * L], f32, tag="ybc4")
                nc.gpsimd.partition_broadcast(ybc4[:], yrow4[0:1, :])

                # mt4[s, L*j + t] = M_{ks[j]}[t, s]
                mt4 = mpool.tile([L, G * L], f32r, tag="mt4")
                nc.vector.tensor_tensor_scan(
                    mt4[:],
                    ybc4[:],
                    ident4[:],
                    0.0,
                    mybir.AluOpType.mult,
                    mybir.AluOpType.add,
                )

                for j, k in enumerate(ks):
                    r0 = k * L
                    tin = tins.pop(k)
                    mtk = mt4[:, L * j : L * j + L]

                    # u[s, :] = x_s * z_s
                    u = upool.tile([L, U], f32r, tag="u")
                    nc.scalar.activation(
                        u[:],
                        tin[:, 2:F],
                        mybir.ActivationFunctionType.Copy,
                        scale=tin[:, 0:1],
                    )

                    po = ps_out_pool.tile([L, U], f32, tag="po")
                    if k > 0:
                        # p_t = prod_{r=block_start..t} y_r = y_0 * mt[0, t]
                        prow = prowpool.tile([1, L], f32, tag="prow")
                        nc.vector.tensor_scalar_mul(
                            prow[:], mtk[0:1, :], tin[0:1, 1:2]
                        )
                        pbc = pbcpool.tile([L, L], f32, tag="pbc")
                        nc.gpsimd.partition_broadcast(pbc[:], prow[0:1, :])
                        # sel[s, t] = I[s==127] * p_t
                        sel = selpool.tile([L, L], bf16, tag="sel")
                        nc.vector.tensor_scalar_mul(sel[:], pbc[:], e127c[:])
                    for jj in (0, 512):
                        nc.tensor.matmul(
                            po[:, jj : jj + 512],
                            mtk,
                            u[:, jj : jj + 512],
                            start=True,
                            stop=(k == 0),
                        )
                    if k > 0:
                        # po[t, :] += p_t * prev[127, :]
                        for jj in (0, 512):
                            nc.tensor.matmul(
                                po[:, jj : jj + 512],
                                sel[:],
                                prev[:, jj : jj + 512],
                                start=False,
                                stop=True,
                            )
                    # bf16 carry copy (feeds the next block's rank-1) first,
                    # full-precision output drain second
                    otb = carrypool.tile([L, U], bf16, tag="otb")
                    nc.scalar.copy(otb[:, 0:512], po[:, 0:512])
                    nc.vector.tensor_copy(otb[:, 512:1024], po[:, 512:1024])
                    ot = outpool.tile([L, U], f32r, tag="ot")
                    nc.scalar.copy(ot[:, 0:512], po[:, 0:512])
                    nc.vector.tensor_copy(ot[:, 512:1024], po[:, 512:1024])
                    nc.sync.dma_start(out[r0 : r0 + L, :], ot[:].bitcast(f32))
                    prev = otb
    nc.finalize()
    return nc


_NC = None


def _get_nc() -> bass.Bass:
    global _NC
    if _NC is None:
        _NC = build_nc()
    return _NC


def kernel(**inputs: np.ndarray) -> np.ndarray:
    x = np.ascontiguousarray(inputs["inputs"], dtype=np.float32)
    assert x.shape == (B, T, F), x.shape
    nc = _get_nc()
    in_maps = [{"inp": x[c]} for c in range(B)]
    res = run_bass_kernel_spmd(nc, in_maps, core_ids=list(range(B)))
    return np.stack([res.results[c]["out"] for c in range(B)], axis=0)
